# revision 28
# baseline (speedup 1.0000x reference)
"""Trainium2 Bass kernel for nn_AppPreUserPGtrDocAttn (sparse_attention).

Strategy: pure data-parallel over the window dim N across 8 NeuronCores.
Each core computes 512 output windows (last core: 509 real + 3 discarded).
All weights are replicated; inputs are sharded/padded/transposed on host.
Compute dtype: fp8 DoubleRow matmuls (2 k-tiles/instruction) with fp32 PSUM
in every stage (A, B and F). All large DRAM tensors are partition-major so
every DMA descriptor line is multi-KB contiguous; small constants are packed
into two blobs (bf16 + fp8) to minimize dma_start count (each dma_start
costs ~0.65us of serial sync-queue dispatch).

Stage A is split into two column halves (x columns 0:264 and 264:512) so
the attention stages for window half 0 (and then stage F's first two window
blocks) overlap the second half of the app-embedding stream. PSUM is
budgeted exactly: pre-pool 1 bank (preamble one-hots), stage A 2 rotating
banks + stage B 6 banks (the halo-column psum reuses a stage-A bank), then
stage F takes all 8.

Scales: x is stored as 16x in fp8 (e4m3 sweet spot); attn_W stays true
scale (tanh input rescaled by 1/16 in the activation), fc_w is 16x so yT
psum is 256x (copies rescale by 1/256); o2 is 64x and dec_w 16x so the
stage-F psum holds logit*1024, stored as fp8 logits*128 (sigmoid on host).
"""

import numpy as np

try:
    import concourse.bass as bass
except ImportError:  # pragma: no cover
    import sys

    sys.path.insert(0, "/opt/trn_rl_repo")
    import concourse.bass as bass

import ml_dtypes

import concourse.mybir as mybir
from concourse import bacc, bass_utils
from concourse import tile
from concourse.tile import TileContext

BF = ml_dtypes.bfloat16
F32 = mybir.dt.float32
BF16 = mybir.dt.bfloat16
FP8 = mybir.dt.float8e4
F8 = ml_dtypes.float8_e4m3
AF = mybir.ActivationFunctionType
ALU = mybir.AluOpType
DR = mybir.MatmulPerfMode.DoubleRow

S = 4096            # sequence length
NWIN = S - 3        # 4093 windows
NCORES = 8
R = 512             # windows per core (last core: 509 real)
RH = R + 3          # x rows needed per core (halo)
RP = 520            # padded col count for xT (512 + 8 halo)
COL0 = 264          # stage-A half-0 x columns (windows 0:256 + reach)
COL1 = R - COL0     # 248
KAPP = 10000        # app vocab / contraction dim
KAPPP = 10240       # padded to 80 k-tiles of 128
NKT = KAPPP // 128  # 80
KB = 20             # k-tiles per DMA batch
NB = NKT // KB      # 4 batches
E = 256             # app emb dim
TE = 64             # tim emb dim
D = 320             # INPUT_SIZE
DP = 384            # padded feature dim (3 k-tiles of 128)
NOUT = 10000        # decoder outputs
NOUTP = 10240       # padded to 20 chunks of 512
GW = 2048           # out cols per group (4 chunks of 512)
NG = NOUTP // GW    # 5
OSCALE = 8.0        # psum (logit*1024) divided by this before fp8 store

# bf16 blob column map ([128, CBLOB])
C_EMBT = 0                     # [48, 64]       emb_tim table
C_TIMV = C_EMBT + 64           # [1, 512]       tim values (p0)
C_PTIMV = C_TIMV + 512         # [1, 512]       ptim values (p0)
CBLOB = C_PTIMV + 512

# fp8 blob column map ([128, CB8])
C8_ATTNW = 0                   # [128, 4*128]   attn_W replicated (t3 zero)
C8_FCW = C8_ATTNW + 512        # [128, 4*256]   fc_w.T * 16 (t3 zero)
C8_HALOA = C8_FCW + 1024       # [128, 16]      halo x*16 app-part (2 mt x 8)
C8_HALOT = C8_HALOA + 16       # [64, 8]        halo x*16 tim-part
CB8 = C8_HALOT + 8

_CACHE: dict = {}


def _build():
    nc = bacc.Bacc()

    appT0_d = nc.declare_dram_parameter("appT0", [128, NKT * COL0], FP8,
                                        isOutput=False)
    appT1_d = nc.declare_dram_parameter("appT1", [128, NKT * COL1], FP8,
                                        isOutput=False)
    wapp_d = nc.declare_dram_parameter("wapp", [128, NKT * E], FP8, isOutput=False)
    decw_d = nc.declare_dram_parameter("decw", [128, NG * 3 * GW], FP8, isOutput=False)
    blob_d = nc.declare_dram_parameter("blob", [128, CBLOB], BF16, isOutput=False)
    blob8_d = nc.declare_dram_parameter("blob8", [128, CB8], FP8, isOutput=False)
    # constf cols: 0 iota, 1:3 uid_emb, 3:5 fc_b, 5:9 attn_b (all [128,1] views)
    constf_d = nc.declare_dram_parameter("constf", [128, 9], F32, isOutput=False)
    out_d = nc.declare_dram_parameter("out", [R, NOUTP], FP8, isOutput=True)

    with TileContext(nc) as tc:
        with (
            tc.tile_pool(name="const", bufs=1) as const,
            tc.tile_pool(name="sb", bufs=1) as sb,
            tc.tile_pool(name="apool0", bufs=4) as apool0,
            tc.tile_pool(name="apool1", bufs=4) as apool1,
            tc.tile_pool(name="wpool", bufs=1) as wpool,
            tc.tile_pool(name="dpool", bufs=1) as dpool,
            tc.tile_pool(name="opool", bufs=3) as opool,
            tc.tile_pool(name="tmp", bufs=1) as tmp,
        ):
            # ---- constants: blob (bf16) + blob8 (fp8) + constf (f32) ----
            blob = const.tile([128, CBLOB], BF16)
            nc.sync.dma_start(blob[:], blob_d[:, :])
            blob8 = const.tile([128, CB8], FP8)
            nc.sync.dma_start(blob8[:], blob8_d[:, :])
            constf_sb = const.tile([128, 9], F32)
            nc.sync.dma_start(constf_sb[:], constf_d[:, :])
            attnwr_sb = blob8[:, C8_ATTNW:C8_ATTNW + 512].rearrange(
                "p (t m) -> p t m", t=4)
            fcw_sb = blob8[:, C8_FCW:C8_FCW + 1024].rearrange(
                "p (t e) -> p t e", t=4)
            embt_sb = blob[0:48, C_EMBT:C_EMBT + TE]
            timv_sb = blob[0:1, C_TIMV:C_TIMV + 512]
            ptimv_sb = blob[0:1, C_PTIMV:C_PTIMV + 512]
            iota_sb = constf_sb[:, 0:1]

            ones_sb = const.tile([1, 128], BF16)
            nc.vector.memset(ones_sb[:], 1.0)

            # persistent activations (xT stored as 16x fp8)
            xTa = sb.tile([128, 2, RP], FP8)       # x.T*16 features 0:256
            xTt = sb.tile([TE, 2, RP], FP8)        # [x.T*16 feat 256:320 | 0]
            H4 = sb.tile([128, R, 4], BF16)        # tanh windows, bcast over P
            rec = sb.tile([128, R], F32)           # 1/L1, bcast over P
            yT = sb.tile([128, 2, RP], BF16)       # fc_w @ x.T (true scale)
            o2a = sb.tile([128, 2, R], FP8)        # out2.T rows 0:256, x64
            o2t = sb.tile([128, 2, R], FP8)        # [zeros | out2.T rows 256:384]

            # halo x columns (host-computed, copied out of the fp8 blob)
            nc.vector.tensor_copy(xTa[:, 0, 512:RP],
                                  blob8[:, C8_HALOA:C8_HALOA + 8])
            nc.vector.tensor_copy(xTa[:, 1, 512:RP],
                                  blob8[:, C8_HALOA + 8:C8_HALOA + 16])
            nc.vector.tensor_copy(xTt[:, 0, 512:RP],
                                  blob8[0:TE, C8_HALOT:C8_HALOT + 8])
            nc.vector.memset(xTt[:, 1, :], 0.0)    # zero half of the DR pair

            # pre-warm ACT tanh table off the critical path
            warm = const.tile([1, 1], F32)
            nc.vector.memset(warm[:], 0.5)
            nc.scalar.activation(warm[:], warm[:], AF.Tanh)

            # ---- tim / ptim one-hot embedding gathers (1 rotating PSUM bank)
            with tc.tile_pool(name="pspre", bufs=1, space="PSUM") as pspre:
                oh = tmp.tile([48, R], BF16, name="oh")
                ohp = tmp.tile([48, R], BF16, name="ohp")

                pre = pspre.tile([TE, 512], F32, name="pre")
                nc.tensor.matmul(pre[0:48, :], ones_sb[0:1, 0:48], timv_sb[:],
                                 start=True, stop=True)
                nc.vector.tensor_scalar(oh[:], pre[0:48, :], iota_sb[0:48, :],
                                        None, op0=ALU.is_equal)
                pre = pspre.tile([TE, 512], F32, name="pre")
                nc.tensor.matmul(pre[:], embt_sb[:], oh[:],
                                 start=True, stop=True)
                nc.vector.tensor_scalar_mul(xTt[:, 0, 0:512], pre[:], 16.0)

                pre = pspre.tile([TE, 512], F32, name="pre")
                nc.tensor.matmul(pre[0:48, :], ones_sb[0:1, 0:48], ptimv_sb[:],
                                 start=True, stop=True)
                nc.vector.tensor_scalar(ohp[:], pre[0:48, :], iota_sb[0:48, :],
                                        None, op0=ALU.is_equal)
                pre = pspre.tile([TE, 512], F32, name="pre")
                nc.tensor.matmul(pre[:], embt_sb[:], ohp[:],
                                 start=True, stop=True)
                nc.vector.memset(o2t[:, 0, :], 0.0)
                nc.scalar.mul(o2t[0:TE, 1, :], pre[:], 64.0)
                nc.vector.memset(o2t[TE:128, 1, :], 0.0)
                nc.vector.memset(o2t[TE:TE + 1, 1, :], 64.0)  # bias row (dec_b)

            # resident emb_app weight tiles (loaded during half 0's stream,
            # interleaved with the at batches; reused by half 1)
            wapp_r = wapp_d.rearrange("p (kt e) -> p kt e", e=E)
            wts = []

            appT_rs = [
                appT0_d.rearrange("p (kt c) -> p kt c", c=COL0),
                appT1_d.rearrange("p (kt c) -> p kt c", c=COL1),
            ]
            NPAIR = NKT // 2
            COLW = [COL0, COL1]

            # ---- stage A: DMA emission keeps the proven v8 stream order
            #      (wt/at-h0 interleaved, then at-h1), but matmul emission is
            #      decoupled so half-1 batches fill the PE idle gaps of the
            #      DMA-paced half-0 phase. PSUM: psA holds the 4 stage-A
            #      accumulators (4 banks) + psB packs pw into two bank-exact
            #      [128,2,256] tiles rotating between window halves and two
            #      full py banks (4 banks). 4+4=8. ----
            with tc.tile_pool(name="psA", bufs=1, space="PSUM") as psA:
                pxa = {(h, mt): psA.tile([128, 280], F32, name=f"pxa{h}{mt}")
                       for h in (0, 1) for mt in (0, 1)}
                at_tiles = {}
                for b in range(NB):
                    wt = wpool.tile([128, KB, E], FP8, name=f"wt{b}")
                    nc.sync.dma_start(wt[:], wapp_r[:, b * KB:(b + 1) * KB, :])
                    wts.append(wt)
                    at = apool0.tile([128, KB, COL0], FP8, name="at")
                    nc.sync.dma_start(at[:],
                                      appT_rs[0][:, b * KB:(b + 1) * KB, :])
                    at_tiles[(0, b)] = at
                for b in range(NB):
                    at = apool1.tile([128, KB, COL1], FP8, name="at")
                    at_dma = nc.sync.dma_start(
                        at[:], appT_rs[1][:, b * KB:(b + 1) * KB, :])
                    at_tiles[(1, b)] = at

                pair = [0, 0]

                def mm_batch(h, b):
                    at = at_tiles[(h, b)]
                    for k in range(0, KB, 2):
                        start = pair[h] == 0
                        stop = pair[h] == NPAIR - 1
                        for mt in range(2):
                            nc.tensor.matmul(
                                pxa[(h, mt)][:, 0:COLW[h]],
                                wts[b][:, k:k + 2, mt * 128:(mt + 1) * 128],
                                at[:, k:k + 2, :],
                                start=start, stop=stop, perf_mode=DR)
                        pair[h] += 1

                for h, b in ((0, 0), (0, 1), (0, 2), (1, 0), (0, 3)):
                    mm_batch(h, b)
                # conversion half 0 (psum already x*16)
                nc.vector.tensor_copy(xTa[:, 0, 0:COL0],
                                      pxa[(0, 0)][:, 0:COL0])
                nc.scalar.copy(xTa[:, 1, 0:COL0], pxa[(0, 1)][:, 0:COL0])

                with tc.tile_pool(name="psB", bufs=1, space="PSUM") as psB:
                    py = [psB.tile([128, 512], F32, name=f"py{mt}")
                          for mt in range(2)]

                    def stage_b_half(wh, pws, py8=None):
                        """fp8 DoubleRow pw/py matmuls for window half wh.
                        k-pairs: (xTa mt0, xTa mt1) then (xTt, zeros). py
                        column ranges match the yT copy split (0:260|260:512).
                        pws = two packed [128,2,256] psum tiles (f0..f3)."""
                        w0 = wh * 256
                        y0, y1 = (0, 260) if wh == 0 else (260, 512)
                        for kp in range(2):
                            if kp == 0:
                                rha, kl = xTa, 128
                            else:
                                rha, kl = xTt, TE
                            st, sp = kp == 0, kp == 1
                            tp = slice(2 * kp, 2 * kp + 2)
                            for f in range(4):
                                nc.tensor.matmul(
                                    pws[f // 2][0:128, f % 2, :],
                                    attnwr_sb[0:kl, tp, :],
                                    rha[0:kl, 0:2, w0 + f:w0 + f + 256],
                                    start=st, stop=sp, perf_mode=DR)
                            for mt in range(2):
                                lhsT = fcw_sb[0:kl, tp,
                                              mt * 128:(mt + 1) * 128]
                                nc.tensor.matmul(py[mt][:, y0:y1], lhsT,
                                                 rha[0:kl, 0:2, y0:y1],
                                                 start=st, stop=sp,
                                                 perf_mode=DR)
                                if py8 is not None:
                                    nc.tensor.matmul(
                                        py8[:, mt * 8:mt * 8 + 8], lhsT,
                                        rha[0:kl, 0:2, 512:RP],
                                        start=st, stop=sp, perf_mode=DR)

                    l1 = tmp.tile([128, R], F32, name="l1")
                    acc = [tmp.tile([128, R], BF16, name=f"acc{mt}")
                           for mt in range(2)]
                    prod = [tmp.tile([128, R], BF16, name=f"prod{mt}")
                            for mt in range(2)]
                    HW = R // 2

                    def stage_cde_half(wh, pws):
                        ws = slice(wh * HW, (wh + 1) * HW)
                        for f in range(4):
                            nc.scalar.activation(H4[:, ws, f],
                                                 pws[f // 2][:, f % 2, :],
                                                 AF.Tanh, scale=1.0 / 16.0,
                                                 bias=constf_sb[:, 5 + f:6 + f])
                        # copy ranges overlap by 4 cols so half-0's shifted
                        # window reads (cols 256:259) stay within half-0
                        ys = slice(0, HW + 4) if wh == 0 else slice(HW + 4, R)
                        nc.vector.tensor_scalar_mul(yT[:, 0, ys],
                                                    py[0][:, ys], 1.0 / 256.0)
                        nc.scalar.mul(yT[:, 1, ys], py[1][:, ys], 1.0 / 256.0)
                        nc.vector.tensor_reduce(
                            l1[:, ws], H4[:, ws, :],
                            mybir.AxisListType.X, ALU.add,
                            apply_absolute_value=True)
                        nc.vector.reciprocal_approx_fast(rec[:, ws],
                                                         l1[:, ws])
                        for mt, eng in ((0, nc.vector), (1, nc.gpsimd)):
                            a, pr = acc[mt], prod[mt]
                            eng.tensor_mul(a[:, ws], H4[:, ws, 0],
                                           yT[:, mt, wh * HW:wh * HW + HW])
                            for f in range(1, 4):
                                eng.tensor_mul(pr[:, ws], H4[:, ws, f],
                                               yT[:, mt, wh * HW + f:
                                                  wh * HW + f + HW])
                                eng.tensor_add(a[:, ws], a[:, ws], pr[:, ws])
                            eng.tensor_mul(a[:, ws], a[:, ws], rec[:, ws])
                            eng.tensor_scalar(o2a[:, mt, ws], a[:, ws],
                                              constf_sb[:, 3 + mt:4 + mt],
                                              constf_sb[:, 1 + mt:2 + mt],
                                              op0=ALU.add, op1=ALU.mult)

                    pws0 = [psB.tile([128, 2, 256], F32, name="pw01"),
                            psB.tile([128, 2, 256], F32, name="pw23")]
                    stage_b_half(0, pws0)
                    stage_cde_half(0, pws0)
                    # half-1's remaining batches run on the PE while the
                    # half-0 attention chain occupies ACT/DVE/GPS
                    for b in (1, 2, 3):
                        mm_batch(1, b)
                    nc.scalar.copy(xTa[:, 0, COL0:R], pxa[(1, 0)][:, 0:COL1])
                    nc.scalar.copy(xTa[:, 1, COL0:R], pxa[(1, 1)][:, 0:COL1])
                    # halo-column fc psum rotates into a freed stage-A bank
                    py8t = psA.tile([128, 280], F32, name="pxa00")
                    pws1 = [psB.tile([128, 2, 256], F32, name="pw01"),
                            psB.tile([128, 2, 256], F32, name="pw23")]
                    stage_b_half(1, pws1, py8=py8t[:, 0:16])
                    nc.vector.tensor_scalar_mul(yT[:, 0, 512:RP],
                                                py8t[:, 0:8], 1.0 / 256.0)
                    nc.scalar.mul(yT[:, 1, 512:RP], py8t[:, 8:16],
                                  1.0 / 256.0)
                    stage_cde_half(1, pws1)

            # ---- stage F: psum = out2T.T @ decw (DoubleRow, 2 instr/chunk);
            #      scaled fp8 logit store; ACT and DVE alternate halves ----
            # bufs=3 (6 banks): depth 3 still hides the conversion latency
            # (3 x 864ns of matmul > ~2us conversion turnaround) and the
            # smaller pool can land on PSUM banks freed earlier by stage A
            with tc.tile_pool(name="psF", bufs=3, space="PSUM") as psF:
                decw_r = decw_d.rearrange("p (g t c) -> p g t c", t=3, c=GW)
                dws = []
                for g in range(NG):
                    dw = dpool.tile([128, 3, GW], FP8, name=f"dw{g}")
                    dw_dma = nc.sync.dma_start(dw[:], decw_r[:, g])
                    tile.add_dep_helper(
                        dw_dma.ins, at_dma.ins, sync=True,
                        reason="defer dec stream until stage-A input stream done")
                    dws.append(dw)
                # two sweeps: window blocks 0-1 (ready after E-half0), then 2-3
                ci = 0
                for mtws in ((0, 1), (2, 3)):
                    for g in range(NG):
                        dw = dws[g]
                        for mtw in mtws:
                            ob = opool.tile([128, GW], FP8, name="ob")
                            for h in range(2):
                                pf = psF.tile([128, 1024], F32, name="pf")
                                for s2 in range(2):
                                    sub = h * 2 + s2
                                    psl = slice(s2 * 512, (s2 + 1) * 512)
                                    sl = slice(sub * 512, (sub + 1) * 512)
                                    nc.tensor.matmul(
                                        pf[:, psl],
                                        o2a[:, :, mtw * 128:(mtw + 1) * 128],
                                        dw[:, 0:2, sl],
                                        start=True, stop=False, perf_mode=DR)
                                    nc.tensor.matmul(
                                        pf[:, psl],
                                        o2t[:, :, mtw * 128:(mtw + 1) * 128],
                                        dw[:, 1:3, sl],
                                        start=False, stop=True, perf_mode=DR)
                                osl = slice(h * 1024, (h + 1) * 1024)
                                if ci % 2 == 0:
                                    nc.scalar.mul(ob[:, osl], pf[:],
                                                  1.0 / OSCALE)
                                else:
                                    nc.vector.tensor_scalar_mul(
                                        ob[:, osl], pf[:], 1.0 / OSCALE)
                                ci += 1
                            nc.sync.dma_start(
                                out_d[mtw * 128:(mtw + 1) * 128,
                                      g * GW:(g + 1) * GW],
                                ob[:])

    nc.finalize()
    return nc


def _host_prep(tim, app, uid, ptim, emb_tim_w, emb_uid_w, emb_app_w,
               attn_W, attn_b, attn_fc_w, attn_fc_b, dec_w, dec_b):
    """Shard + pad + transpose + cast all inputs; returns in_maps for 8 cores."""
    app = np.asarray(app, dtype=np.float32)
    tim = np.asarray(tim).reshape(-1)
    ptim = np.asarray(ptim).reshape(-1)
    uid = int(np.asarray(uid).reshape(-1)[0])

    app_f8 = app.astype(F8)

    wapp_lin = np.zeros((KAPPP, E), dtype=F8)
    wapp_lin[:KAPP] = (np.asarray(emb_app_w, dtype=np.float32) * 16.0).astype(F8)
    wapp = np.ascontiguousarray(
        wapp_lin.reshape(NKT, 128, E).transpose(1, 0, 2).reshape(128, NKT * E))
    wapp_f32 = wapp_lin.astype(np.float32)

    decw_lin = np.zeros((DP, NOUTP), dtype=F8)
    dwT = np.ascontiguousarray(np.asarray(dec_w, dtype=np.float32).T)  # [320, 10000]
    decw_lin[:D, :NOUT] = (dwT * 16.0).astype(F8)
    decw_lin[D, :NOUT] = (np.asarray(dec_b, dtype=np.float32) * 16.0).astype(F8)
    decw = np.ascontiguousarray(
        decw_lin.reshape(3, 128, NG, GW).transpose(1, 2, 0, 3)
        .reshape(128, NG * 3 * GW))

    embt = np.asarray(emb_tim_w, dtype=np.float32).astype(BF)

    # bf16 blob: embt | timv | ptimv
    blob0 = np.zeros((128, CBLOB), dtype=BF)
    blob0[0:48, C_EMBT:C_EMBT + TE] = embt

    # fp8 blob: attnwr (true scale) | fcw*16 | halo*16
    blob8 = np.zeros((128, CB8), dtype=F8)
    attnw = np.zeros((DP,), dtype=np.float32)
    attnw[:D] = np.asarray(attn_W, dtype=np.float32).reshape(-1)
    aw = np.repeat(attnw[:, None], 128, axis=1)          # [384, 128]
    blob8[:, C8_ATTNW:C8_ATTNW + 384] = (
        aw.reshape(3, 128, 128).transpose(1, 0, 2).reshape(128, 384)).astype(F8)
    fcw_lin = np.zeros((DP, E), dtype=np.float32)
    fcw_lin[:D] = np.asarray(attn_fc_w, dtype=np.float32).T * 16.0
    blob8[:, C8_FCW:C8_FCW + 768] = (
        fcw_lin.reshape(3, 128, E).transpose(1, 0, 2).reshape(128, 768)).astype(F8)

    uide = np.asarray(emb_uid_w, dtype=np.float32)[uid]
    fcb = np.asarray(attn_fc_b, dtype=np.float32).reshape(-1)
    constf = np.zeros((128, 9), dtype=np.float32)
    constf[:, 0] = np.arange(128, dtype=np.float32)
    constf[:, 1] = uide[0:128] * 64.0
    constf[:, 2] = uide[128:256] * 64.0
    constf[:, 3] = fcb[0:128]
    constf[:, 4] = fcb[128:256]
    constf[:, 5:9] = np.asarray(attn_b, dtype=np.float32).reshape(1, 4)

    in_maps = []
    for c in range(NCORES):
        r0 = c * R
        appT_lin = np.zeros((KAPPP, R), dtype=F8)
        appT_lin[:KAPP] = app_f8[r0:r0 + R].T
        appT0 = np.ascontiguousarray(
            appT_lin[:, 0:COL0].reshape(NKT, 128, COL0)
            .transpose(1, 0, 2).reshape(128, NKT * COL0))
        appT1 = np.ascontiguousarray(
            appT_lin[:, COL0:R].reshape(NKT, 128, COL1)
            .transpose(1, 0, 2).reshape(128, NKT * COL1))

        blob = blob0.copy()
        blob[0:1, C_TIMV:C_TIMV + 512] = tim[r0:r0 + R].astype(BF)
        np_ = min(r0 + R, NWIN) - r0
        blob[0:1, C_PTIMV:C_PTIMV + np_] = ptim[r0:r0 + np_].astype(BF)

        b8 = blob8.copy()
        nh = min(3, S - (r0 + R)) if r0 + R < S else 0
        if nh > 0:
            rows = app_f8[r0 + R:r0 + R + nh].astype(np.float32)
            xh = rows @ wapp_f32[:KAPP]                    # [nh, 256] = x*16
            b8[:, C8_HALOA:C8_HALOA + nh] = xh.T[0:128].astype(F8)
            b8[:, C8_HALOA + 8:C8_HALOA + 8 + nh] = xh.T[128:256].astype(F8)
            b8[0:TE, C8_HALOT:C8_HALOT + nh] = (
                embt[tim[r0 + R:r0 + R + nh]].astype(np.float32) * 16.0
            ).T.astype(F8)

        in_maps.append({
            "appT0": appT0, "appT1": appT1, "wapp": wapp, "decw": decw,
            "blob": blob, "blob8": b8, "constf": constf,
        })
    return in_maps


def kernel(tim, app, loc, uid, ptim, emb_tim_w, emb_uid_w, emb_app_w,
           attn_W, attn_b, attn_fc_w, attn_fc_b, dec_w, dec_b,
           _trace=False, _trace_kwargs=None):
    if "nc" not in _CACHE:
        _CACHE["nc"] = _build()
    nc = _CACHE["nc"]

    in_maps = _host_prep(tim, app, uid, ptim, emb_tim_w, emb_uid_w, emb_app_w,
                         attn_W, attn_b, attn_fc_w, attn_fc_b, dec_w, dec_b)

    kw = {}
    if _trace:
        kw["trace"] = True
        if _trace_kwargs:
            kw.update(_trace_kwargs)
    res = bass_utils.run_bass_kernel_spmd(nc, in_maps, core_ids=list(range(NCORES)), **kw)
    _CACHE["last_result"] = res

    outs = []
    for c in range(NCORES):
        nrows = R if c < NCORES - 1 else NWIN - (NCORES - 1) * R
        outs.append(np.asarray(res.results[c]["out"])[:nrows, :NOUT])
    logits = np.concatenate(outs, axis=0).astype(np.float32) * (OSCALE / 1024.0)
    return 1.0 / (1.0 + np.exp(-logits))


# revision 30
# speedup vs baseline: 1.0189x; 1.0189x over previous
"""Trainium2 Bass kernel for nn_AppPreUserPGtrDocAttn (sparse_attention).

Strategy: pure data-parallel over the window dim N across 8 NeuronCores.
Each core computes 512 output windows (last core: 509 real + 3 discarded).
All weights are replicated; inputs are sharded/padded/transposed on host.
Compute dtype: fp8 DoubleRow matmuls (2 k-tiles/instruction) with fp32 PSUM
in every stage (A, B and F). All large DRAM tensors are partition-major so
every DMA descriptor line is multi-KB contiguous; small constants are packed
into two blobs (bf16 + fp8) to minimize dma_start count (each dma_start
costs ~0.65us of serial sync-queue dispatch).

Stage A is split into two column halves (x columns 0:264 and 264:512) so
the attention stages for window half 0 (and then stage F's first two window
blocks) overlap the second half of the app-embedding stream. PSUM is
budgeted exactly: pre-pool 1 bank (preamble one-hots), stage A 2 rotating
banks + stage B 6 banks (the halo-column psum reuses a stage-A bank), then
stage F takes all 8.

Scales: x is stored as 16x in fp8 (e4m3 sweet spot); attn_W stays true
scale (tanh input rescaled by 1/16 in the activation), fc_w is 16x so yT
psum is 256x (copies rescale by 1/256); o2 is 64x and dec_w 16x so the
stage-F psum holds logit*1024, stored as fp8 logits*128 (sigmoid on host).
"""

import numpy as np

try:
    import concourse.bass as bass
except ImportError:  # pragma: no cover
    import sys

    sys.path.insert(0, "/opt/trn_rl_repo")
    import concourse.bass as bass

import ml_dtypes

import concourse.mybir as mybir
from concourse import bacc, bass_utils
from concourse import tile
from concourse.tile import TileContext

BF = ml_dtypes.bfloat16
F32 = mybir.dt.float32
BF16 = mybir.dt.bfloat16
FP8 = mybir.dt.float8e4
F8 = ml_dtypes.float8_e4m3
AF = mybir.ActivationFunctionType
ALU = mybir.AluOpType
DR = mybir.MatmulPerfMode.DoubleRow

S = 4096            # sequence length
NWIN = S - 3        # 4093 windows
NCORES = 8
R = 512             # windows per core (last core: 509 real)
RH = R + 3          # x rows needed per core (halo)
RP = 520            # padded col count for xT (512 + 8 halo)
COL0 = 264          # stage-A half-0 x columns (windows 0:256 + reach)
COL1 = R - COL0     # 248
KAPP = 10000        # app vocab / contraction dim
KAPPP = 10240       # padded to 80 k-tiles of 128
NKT = KAPPP // 128  # 80
KB = 20             # k-tiles per DMA batch
NB = NKT // KB      # 4 batches
E = 256             # app emb dim
TE = 64             # tim emb dim
D = 320             # INPUT_SIZE
DP = 384            # padded feature dim (3 k-tiles of 128)
NOUT = 10000        # decoder outputs
NOUTP = 10240       # padded to 20 chunks of 512
GW = 2048           # out cols per group (4 chunks of 512)
NG = NOUTP // GW    # 5
OSCALE = 8.0        # psum (logit*1024) divided by this before fp8 store

# bf16 blob column map ([128, CBLOB])
C_EMBT = 0                     # [48, 64]       emb_tim table
C_TIMV = C_EMBT + 64           # [1, 512]       tim values (p0)
C_PTIMV = C_TIMV + 512         # [1, 512]       ptim values (p0)
CBLOB = C_PTIMV + 512

# fp8 blob column map ([128, CB8])
C8_ATTNW = 0                   # [128, 4*128]   attn_W replicated (t3 zero)
C8_FCW = C8_ATTNW + 512        # [128, 4*256]   fc_w.T * 16 (t3 zero)
C8_HALOA = C8_FCW + 1024       # [128, 16]      halo x*16 app-part (2 mt x 8)
C8_HALOT = C8_HALOA + 16       # [64, 8]        halo x*16 tim-part
CB8 = C8_HALOT + 8

_CACHE: dict = {}


def _build():
    nc = bacc.Bacc()

    appT0_d = nc.declare_dram_parameter("appT0", [128, NKT * COL0], FP8,
                                        isOutput=False)
    appT1_d = nc.declare_dram_parameter("appT1", [128, NKT * COL1], FP8,
                                        isOutput=False)
    wapp_d = nc.declare_dram_parameter("wapp", [128, NKT * E], FP8, isOutput=False)
    decw_d = nc.declare_dram_parameter("decw", [128, NG * 3 * GW], FP8, isOutput=False)
    blob_d = nc.declare_dram_parameter("blob", [128, CBLOB], BF16, isOutput=False)
    blob8_d = nc.declare_dram_parameter("blob8", [128, CB8], FP8, isOutput=False)
    # constf cols: 0 iota, 1:3 uid_emb, 3:5 fc_b, 5:9 attn_b (all [128,1] views)
    constf_d = nc.declare_dram_parameter("constf", [128, 9], F32, isOutput=False)
    out_d = nc.declare_dram_parameter("out", [R, NOUTP], FP8, isOutput=True)

    with TileContext(nc) as tc:
        with (
            tc.tile_pool(name="const", bufs=1) as const,
            tc.tile_pool(name="sb", bufs=1) as sb,
            tc.tile_pool(name="apool0", bufs=4) as apool0,
            tc.tile_pool(name="apool1", bufs=4) as apool1,
            tc.tile_pool(name="wpool", bufs=1) as wpool,
            tc.tile_pool(name="dpool", bufs=1) as dpool,
            tc.tile_pool(name="opool", bufs=3) as opool,
            tc.tile_pool(name="tmp", bufs=1) as tmp,
        ):
            # ---- constants: blob (bf16) + blob8 (fp8) + constf (f32) ----
            blob = const.tile([128, CBLOB], BF16)
            nc.sync.dma_start(blob[:], blob_d[:, :])
            blob8 = const.tile([128, CB8], FP8)
            nc.sync.dma_start(blob8[:], blob8_d[:, :])
            constf_sb = const.tile([128, 9], F32)
            nc.sync.dma_start(constf_sb[:], constf_d[:, :])
            attnwr_sb = blob8[:, C8_ATTNW:C8_ATTNW + 512].rearrange(
                "p (t m) -> p t m", t=4)
            fcw_sb = blob8[:, C8_FCW:C8_FCW + 1024].rearrange(
                "p (t e) -> p t e", t=4)
            embt_sb = blob[0:48, C_EMBT:C_EMBT + TE]
            timv_sb = blob[0:1, C_TIMV:C_TIMV + 512]
            ptimv_sb = blob[0:1, C_PTIMV:C_PTIMV + 512]
            iota_sb = constf_sb[:, 0:1]

            ones_sb = const.tile([1, 128], BF16)
            nc.vector.memset(ones_sb[:], 1.0)

            # persistent activations (xT stored as 16x fp8)
            xTa = sb.tile([128, 2, RP], FP8)       # x.T*16 features 0:256
            xTt = sb.tile([TE, 2, RP], FP8)        # [x.T*16 feat 256:320 | 0]
            H4 = sb.tile([128, R, 4], BF16)        # tanh windows, bcast over P
            rec = sb.tile([128, R], F32)           # 1/L1, bcast over P
            yT = sb.tile([128, 2, RP], BF16)       # fc_w @ x.T (true scale)
            o2a = sb.tile([128, 2, R], FP8)        # out2.T rows 0:256, x64
            o2t = sb.tile([128, 2, R], FP8)        # [zeros | out2.T rows 256:384]

            # halo x columns (host-computed, copied out of the fp8 blob)
            nc.vector.tensor_copy(xTa[:, 0, 512:RP],
                                  blob8[:, C8_HALOA:C8_HALOA + 8])
            nc.vector.tensor_copy(xTa[:, 1, 512:RP],
                                  blob8[:, C8_HALOA + 8:C8_HALOA + 16])
            nc.vector.tensor_copy(xTt[:, 0, 512:RP],
                                  blob8[0:TE, C8_HALOT:C8_HALOT + 8])
            nc.vector.memset(xTt[:, 1, :], 0.0)    # zero half of the DR pair

            # pre-warm ACT tanh table off the critical path
            warm = const.tile([1, 1], F32)
            nc.vector.memset(warm[:], 0.5)
            nc.scalar.activation(warm[:], warm[:], AF.Tanh)

            # ---- tim / ptim one-hot embedding gathers (1 rotating PSUM bank)
            with tc.tile_pool(name="pspre", bufs=1, space="PSUM") as pspre:
                oh = tmp.tile([48, R], BF16, name="oh")
                ohp = tmp.tile([48, R], BF16, name="ohp")

                pre = pspre.tile([TE, 512], F32, name="pre")
                nc.tensor.matmul(pre[0:48, :], ones_sb[0:1, 0:48], timv_sb[:],
                                 start=True, stop=True)
                nc.vector.tensor_scalar(oh[:], pre[0:48, :], iota_sb[0:48, :],
                                        None, op0=ALU.is_equal)
                pre = pspre.tile([TE, 512], F32, name="pre")
                nc.tensor.matmul(pre[:], embt_sb[:], oh[:],
                                 start=True, stop=True)
                nc.vector.tensor_scalar_mul(xTt[:, 0, 0:512], pre[:], 16.0)

                pre = pspre.tile([TE, 512], F32, name="pre")
                nc.tensor.matmul(pre[0:48, :], ones_sb[0:1, 0:48], ptimv_sb[:],
                                 start=True, stop=True)
                nc.vector.tensor_scalar(ohp[:], pre[0:48, :], iota_sb[0:48, :],
                                        None, op0=ALU.is_equal)
                pre = pspre.tile([TE, 512], F32, name="pre")
                nc.tensor.matmul(pre[:], embt_sb[:], ohp[:],
                                 start=True, stop=True)
                nc.vector.memset(o2t[:, 0, :], 0.0)
                nc.scalar.mul(o2t[0:TE, 1, :], pre[:], 64.0)
                nc.vector.memset(o2t[TE:128, 1, :], 0.0)
                nc.vector.memset(o2t[TE:TE + 1, 1, :], 64.0)  # bias row (dec_b)

            # resident emb_app weight tiles (loaded during half 0's stream,
            # interleaved with the at batches; reused by half 1)
            wapp_r = wapp_d.rearrange("p (kt e) -> p kt e", e=E)
            wts = []

            appT_rs = [
                appT0_d.rearrange("p (kt c) -> p kt c", c=COL0),
                appT1_d.rearrange("p (kt c) -> p kt c", c=COL1),
            ]
            NPAIR = NKT // 2
            COLW = [COL0, COL1]

            # ---- stage A: DMA emission keeps the proven v8 stream order
            #      (wt/at-h0 interleaved, then at-h1), but matmul emission is
            #      decoupled so half-1 batches fill the PE idle gaps of the
            #      DMA-paced half-0 phase. PSUM: psA holds the 4 stage-A
            #      accumulators (4 banks) + psB packs pw into two bank-exact
            #      [128,2,256] tiles rotating between window halves and two
            #      full py banks (4 banks). 4+4=8. ----
            with tc.tile_pool(name="psA", bufs=1, space="PSUM") as psA:
                pxa = {(h, mt): psA.tile([128, 280], F32, name=f"pxa{h}{mt}")
                       for h in (0, 1) for mt in (0, 1)}
                at_tiles = {}
                for b in range(NB):
                    wt = wpool.tile([128, KB, E], FP8, name=f"wt{b}")
                    nc.sync.dma_start(wt[:], wapp_r[:, b * KB:(b + 1) * KB, :])
                    wts.append(wt)
                    at = apool0.tile([128, KB, COL0], FP8, name="at")
                    nc.sync.dma_start(at[:],
                                      appT_rs[0][:, b * KB:(b + 1) * KB, :])
                    at_tiles[(0, b)] = at
                for b in range(NB):
                    at = apool1.tile([128, KB, COL1], FP8, name="at")
                    at_dma = nc.sync.dma_start(
                        at[:], appT_rs[1][:, b * KB:(b + 1) * KB, :])
                    at_tiles[(1, b)] = at

                pair = [0, 0]

                def mm_batch(h, b):
                    at = at_tiles[(h, b)]
                    for k in range(0, KB, 2):
                        start = pair[h] == 0
                        stop = pair[h] == NPAIR - 1
                        for mt in range(2):
                            nc.tensor.matmul(
                                pxa[(h, mt)][:, 0:COLW[h]],
                                wts[b][:, k:k + 2, mt * 128:(mt + 1) * 128],
                                at[:, k:k + 2, :],
                                start=start, stop=stop, perf_mode=DR)
                        pair[h] += 1

                for h, b in ((0, 0), (0, 1), (0, 2), (1, 0), (0, 3)):
                    mm_batch(h, b)
                # conversion half 0 (psum already x*16)
                nc.vector.tensor_copy(xTa[:, 0, 0:COL0],
                                      pxa[(0, 0)][:, 0:COL0])
                nc.scalar.copy(xTa[:, 1, 0:COL0], pxa[(0, 1)][:, 0:COL0])

                with tc.tile_pool(name="psB", bufs=1, space="PSUM") as psB:
                    py = [psB.tile([128, 512], F32, name=f"py{mt}")
                          for mt in range(2)]

                    def stage_b_half(wh, pws, py8=None):
                        """fp8 DoubleRow pw/py matmuls for window half wh.
                        k-pairs: (xTa mt0, xTa mt1) then (xTt, zeros). py
                        column ranges match the yT copy split (0:260|260:512).
                        pws = two packed [128,2,256] psum tiles (f0..f3)."""
                        w0 = wh * 256
                        y0, y1 = (0, 260) if wh == 0 else (260, 512)
                        for kp in range(2):
                            if kp == 0:
                                rha, kl = xTa, 128
                            else:
                                rha, kl = xTt, TE
                            st, sp = kp == 0, kp == 1
                            tp = slice(2 * kp, 2 * kp + 2)
                            for f in range(4):
                                nc.tensor.matmul(
                                    pws[f // 2][0:128, f % 2, :],
                                    attnwr_sb[0:kl, tp, :],
                                    rha[0:kl, 0:2, w0 + f:w0 + f + 256],
                                    start=st, stop=sp, perf_mode=DR)
                            for mt in range(2):
                                lhsT = fcw_sb[0:kl, tp,
                                              mt * 128:(mt + 1) * 128]
                                nc.tensor.matmul(py[mt][:, y0:y1], lhsT,
                                                 rha[0:kl, 0:2, y0:y1],
                                                 start=st, stop=sp,
                                                 perf_mode=DR)
                                if py8 is not None:
                                    nc.tensor.matmul(
                                        py8[:, mt * 8:mt * 8 + 8], lhsT,
                                        rha[0:kl, 0:2, 512:RP],
                                        start=st, stop=sp, perf_mode=DR)

                    l1 = tmp.tile([128, R], F32, name="l1")
                    acc = [tmp.tile([128, R], BF16, name=f"acc{mt}")
                           for mt in range(2)]
                    prod = [tmp.tile([128, R], BF16, name=f"prod{mt}")
                            for mt in range(2)]
                    HW = R // 2

                    def stage_cde_half(wh, pws):
                        ws = slice(wh * HW, (wh + 1) * HW)
                        for f in range(4):
                            nc.scalar.activation(H4[:, ws, f],
                                                 pws[f // 2][:, f % 2, :],
                                                 AF.Tanh, scale=1.0 / 16.0,
                                                 bias=constf_sb[:, 5 + f:6 + f])
                        # copy ranges overlap by 4 cols so half-0's shifted
                        # window reads (cols 256:259) stay within half-0
                        ys = slice(0, HW + 4) if wh == 0 else slice(HW + 4, R)
                        nc.vector.tensor_scalar_mul(yT[:, 0, ys],
                                                    py[0][:, ys], 1.0 / 256.0)
                        nc.scalar.mul(yT[:, 1, ys], py[1][:, ys], 1.0 / 256.0)
                        nc.vector.tensor_reduce(
                            l1[:, ws], H4[:, ws, :],
                            mybir.AxisListType.X, ALU.add,
                            apply_absolute_value=True)
                        nc.vector.reciprocal_approx_fast(rec[:, ws],
                                                         l1[:, ws])
                        for mt, eng in ((0, nc.vector), (1, nc.gpsimd)):
                            a, pr = acc[mt], prod[mt]
                            eng.tensor_mul(a[:, ws], H4[:, ws, 0],
                                           yT[:, mt, wh * HW:wh * HW + HW])
                            for f in range(1, 4):
                                eng.tensor_mul(pr[:, ws], H4[:, ws, f],
                                               yT[:, mt, wh * HW + f:
                                                  wh * HW + f + HW])
                                eng.tensor_add(a[:, ws], a[:, ws], pr[:, ws])
                            eng.tensor_mul(a[:, ws], a[:, ws], rec[:, ws])
                            eng.tensor_scalar(o2a[:, mt, ws], a[:, ws],
                                              constf_sb[:, 3 + mt:4 + mt],
                                              constf_sb[:, 1 + mt:2 + mt],
                                              op0=ALU.add, op1=ALU.mult)

                    pws0 = [psB.tile([128, 2, 256], F32, name="pw01"),
                            psB.tile([128, 2, 256], F32, name="pw23")]
                    stage_b_half(0, pws0)
                    stage_cde_half(0, pws0)
                    # half-1's remaining batches run on the PE while the
                    # half-0 attention chain occupies ACT/DVE/GPS
                    for b in (1, 2, 3):
                        mm_batch(1, b)
                    nc.scalar.copy(xTa[:, 0, COL0:R], pxa[(1, 0)][:, 0:COL1])
                    nc.scalar.copy(xTa[:, 1, COL0:R], pxa[(1, 1)][:, 0:COL1])
                    # halo-column fc psum rotates into a freed stage-A bank
                    py8t = psA.tile([128, 280], F32, name="pxa00")
                    pws1 = [psB.tile([128, 2, 256], F32, name="pw01"),
                            psB.tile([128, 2, 256], F32, name="pw23")]
                    stage_b_half(1, pws1, py8=py8t[:, 0:16])
                    nc.vector.tensor_scalar_mul(yT[:, 0, 512:RP],
                                                py8t[:, 0:8], 1.0 / 256.0)
                    nc.scalar.mul(yT[:, 1, 512:RP], py8t[:, 8:16],
                                  1.0 / 256.0)
                    stage_cde_half(1, pws1)
                    # keep the PE clock ramped through the psB->psF pool
                    # drain (~2.4us idle) so stage F's first matmuls run at
                    # full p-state: throwaway DoubleRow matmuls into the
                    # unused columns of the live halo psum bank
                    for _ in range(16):
                        nc.tensor.matmul(py8t[:, 24:280],
                                         o2t[:, :, 0:128],
                                         xTa[:, 0:2, 0:256],
                                         start=True, stop=True, perf_mode=DR)

            # ---- stage F: psum = out2T.T @ decw (DoubleRow, 2 instr/chunk);
            #      scaled fp8 logit store; ACT and DVE alternate halves ----
            with tc.tile_pool(name="psF", bufs=4, space="PSUM") as psF:
                decw_r = decw_d.rearrange("p (g t c) -> p g t c", t=3, c=GW)
                dws = []
                for g in range(NG):
                    dw = dpool.tile([128, 3, GW], FP8, name=f"dw{g}")
                    dw_dma = nc.sync.dma_start(dw[:], decw_r[:, g])
                    tile.add_dep_helper(
                        dw_dma.ins, at_dma.ins, sync=True,
                        reason="defer dec stream until stage-A input stream done")
                    dws.append(dw)
                # two sweeps: window blocks 0-1 (ready after E-half0), then 2-3
                ci = 0
                for mtws in ((0, 1), (2, 3)):
                    for g in range(NG):
                        dw = dws[g]
                        for mtw in mtws:
                            ob = opool.tile([128, GW], FP8, name="ob")
                            for h in range(2):
                                pf = psF.tile([128, 1024], F32, name="pf")
                                for s2 in range(2):
                                    sub = h * 2 + s2
                                    psl = slice(s2 * 512, (s2 + 1) * 512)
                                    sl = slice(sub * 512, (sub + 1) * 512)
                                    nc.tensor.matmul(
                                        pf[:, psl],
                                        o2a[:, :, mtw * 128:(mtw + 1) * 128],
                                        dw[:, 0:2, sl],
                                        start=True, stop=False, perf_mode=DR)
                                    nc.tensor.matmul(
                                        pf[:, psl],
                                        o2t[:, :, mtw * 128:(mtw + 1) * 128],
                                        dw[:, 1:3, sl],
                                        start=False, stop=True, perf_mode=DR)
                                osl = slice(h * 1024, (h + 1) * 1024)
                                if ci % 2 == 0:
                                    nc.scalar.mul(ob[:, osl], pf[:],
                                                  1.0 / OSCALE)
                                else:
                                    nc.vector.tensor_scalar_mul(
                                        ob[:, osl], pf[:], 1.0 / OSCALE)
                                ci += 1
                            nc.sync.dma_start(
                                out_d[mtw * 128:(mtw + 1) * 128,
                                      g * GW:(g + 1) * GW],
                                ob[:])

    nc.finalize()
    return nc


def _host_prep(tim, app, uid, ptim, emb_tim_w, emb_uid_w, emb_app_w,
               attn_W, attn_b, attn_fc_w, attn_fc_b, dec_w, dec_b):
    """Shard + pad + transpose + cast all inputs; returns in_maps for 8 cores."""
    app = np.asarray(app, dtype=np.float32)
    tim = np.asarray(tim).reshape(-1)
    ptim = np.asarray(ptim).reshape(-1)
    uid = int(np.asarray(uid).reshape(-1)[0])

    app_f8 = app.astype(F8)

    wapp_lin = np.zeros((KAPPP, E), dtype=F8)
    wapp_lin[:KAPP] = (np.asarray(emb_app_w, dtype=np.float32) * 16.0).astype(F8)
    wapp = np.ascontiguousarray(
        wapp_lin.reshape(NKT, 128, E).transpose(1, 0, 2).reshape(128, NKT * E))
    wapp_f32 = wapp_lin.astype(np.float32)

    decw_lin = np.zeros((DP, NOUTP), dtype=F8)
    dwT = np.ascontiguousarray(np.asarray(dec_w, dtype=np.float32).T)  # [320, 10000]
    decw_lin[:D, :NOUT] = (dwT * 16.0).astype(F8)
    decw_lin[D, :NOUT] = (np.asarray(dec_b, dtype=np.float32) * 16.0).astype(F8)
    decw = np.ascontiguousarray(
        decw_lin.reshape(3, 128, NG, GW).transpose(1, 2, 0, 3)
        .reshape(128, NG * 3 * GW))

    embt = np.asarray(emb_tim_w, dtype=np.float32).astype(BF)

    # bf16 blob: embt | timv | ptimv
    blob0 = np.zeros((128, CBLOB), dtype=BF)
    blob0[0:48, C_EMBT:C_EMBT + TE] = embt

    # fp8 blob: attnwr (true scale) | fcw*16 | halo*16
    blob8 = np.zeros((128, CB8), dtype=F8)
    attnw = np.zeros((DP,), dtype=np.float32)
    attnw[:D] = np.asarray(attn_W, dtype=np.float32).reshape(-1)
    aw = np.repeat(attnw[:, None], 128, axis=1)          # [384, 128]
    blob8[:, C8_ATTNW:C8_ATTNW + 384] = (
        aw.reshape(3, 128, 128).transpose(1, 0, 2).reshape(128, 384)).astype(F8)
    fcw_lin = np.zeros((DP, E), dtype=np.float32)
    fcw_lin[:D] = np.asarray(attn_fc_w, dtype=np.float32).T * 16.0
    blob8[:, C8_FCW:C8_FCW + 768] = (
        fcw_lin.reshape(3, 128, E).transpose(1, 0, 2).reshape(128, 768)).astype(F8)

    uide = np.asarray(emb_uid_w, dtype=np.float32)[uid]
    fcb = np.asarray(attn_fc_b, dtype=np.float32).reshape(-1)
    constf = np.zeros((128, 9), dtype=np.float32)
    constf[:, 0] = np.arange(128, dtype=np.float32)
    constf[:, 1] = uide[0:128] * 64.0
    constf[:, 2] = uide[128:256] * 64.0
    constf[:, 3] = fcb[0:128]
    constf[:, 4] = fcb[128:256]
    constf[:, 5:9] = np.asarray(attn_b, dtype=np.float32).reshape(1, 4)

    in_maps = []
    for c in range(NCORES):
        r0 = c * R
        appT_lin = np.zeros((KAPPP, R), dtype=F8)
        appT_lin[:KAPP] = app_f8[r0:r0 + R].T
        appT0 = np.ascontiguousarray(
            appT_lin[:, 0:COL0].reshape(NKT, 128, COL0)
            .transpose(1, 0, 2).reshape(128, NKT * COL0))
        appT1 = np.ascontiguousarray(
            appT_lin[:, COL0:R].reshape(NKT, 128, COL1)
            .transpose(1, 0, 2).reshape(128, NKT * COL1))

        blob = blob0.copy()
        blob[0:1, C_TIMV:C_TIMV + 512] = tim[r0:r0 + R].astype(BF)
        np_ = min(r0 + R, NWIN) - r0
        blob[0:1, C_PTIMV:C_PTIMV + np_] = ptim[r0:r0 + np_].astype(BF)

        b8 = blob8.copy()
        nh = min(3, S - (r0 + R)) if r0 + R < S else 0
        if nh > 0:
            rows = app_f8[r0 + R:r0 + R + nh].astype(np.float32)
            xh = rows @ wapp_f32[:KAPP]                    # [nh, 256] = x*16
            b8[:, C8_HALOA:C8_HALOA + nh] = xh.T[0:128].astype(F8)
            b8[:, C8_HALOA + 8:C8_HALOA + 8 + nh] = xh.T[128:256].astype(F8)
            b8[0:TE, C8_HALOT:C8_HALOT + nh] = (
                embt[tim[r0 + R:r0 + R + nh]].astype(np.float32) * 16.0
            ).T.astype(F8)

        in_maps.append({
            "appT0": appT0, "appT1": appT1, "wapp": wapp, "decw": decw,
            "blob": blob, "blob8": b8, "constf": constf,
        })
    return in_maps


def kernel(tim, app, loc, uid, ptim, emb_tim_w, emb_uid_w, emb_app_w,
           attn_W, attn_b, attn_fc_w, attn_fc_b, dec_w, dec_b,
           _trace=False, _trace_kwargs=None):
    if "nc" not in _CACHE:
        _CACHE["nc"] = _build()
    nc = _CACHE["nc"]

    in_maps = _host_prep(tim, app, uid, ptim, emb_tim_w, emb_uid_w, emb_app_w,
                         attn_W, attn_b, attn_fc_w, attn_fc_b, dec_w, dec_b)

    kw = {}
    if _trace:
        kw["trace"] = True
        if _trace_kwargs:
            kw.update(_trace_kwargs)
    res = bass_utils.run_bass_kernel_spmd(nc, in_maps, core_ids=list(range(NCORES)), **kw)
    _CACHE["last_result"] = res

    outs = []
    for c in range(NCORES):
        nrows = R if c < NCORES - 1 else NWIN - (NCORES - 1) * R
        outs.append(np.asarray(res.results[c]["out"])[:nrows, :NOUT])
    logits = np.concatenate(outs, axis=0).astype(np.float32) * (OSCALE / 1024.0)
    return 1.0 / (1.0 + np.exp(-logits))
